# revision 11
# baseline (speedup 1.0000x reference)
"""Trainium2 Bass kernel for nn_HardwiredAttention (NRI-style GNN message passing).

Math (derived from the reference):
  adj[b,t,i,j] = 1/(||locs[b,i,t]-locs[b,j,t]|| + eps) for i!=j, ~0 on diag
  out[b,:,t,:] = adj[b,t] @ hidden[b,:,t,:]          ([48,48] @ [48,128] per (b,t))

Distribution: data-parallel over batch, 8 cores; the batch is processed in
NCHUNK pipelined chunks of 8 (one batch per core per call) so host quant /
dequant and device exec overlap the host<->device link transfers.

The end-to-end call is dominated by the host<->device link (~45 MB/s tunnel),
so the kernel minimizes bytes moved:
  - hidden is int8-quantized host-side with a per-(b,n,t) fp16 scale; the
    scale is folded into the adjacency weights on device (w'_ij = w_ij * s_j),
    so the device only does an int8->fp16 convert on the hidden payload.
  - the output is int8-quantized on device (per-(b,i,t) amax over H via
    reduce_max(abs), round-to-nearest saturating convert) and dequantized
    host-side. The fp16 amax scales are packed into two spare T-rows of the
    same int8 output tensor, so one fetch returns everything.
  - the diag mask / PE-transpose identity are baked into the NEFF as Const
    tensors (no per-call transfer), and no zero output buffers are donated
    (the kernel writes every output element).
  - the jitted shard_map callable is built once and cached.

Per-core device layout (same skeleton as the fp16 baseline):
  - elementwise pipeline in partitions p=(s,tau), t=2*tau+s (100 partitions):
    dx/dy from a tiny [100,(c,n)] coords tile via stride-0 broadcast APs,
    squares on ACT, d2-add on GPSIMD, sqrt on ACT, (d+eps)+BIGMASK via
    scalar_tensor_tensor, reciprocal_approx_fast, then *hscale -> fp16 adj.
  - PE transposes [50(tau),48(j)] -> [48(j),50(tau)] per (i,s) build a
    block-diagonal fp16 lhsT [128=(s,j), (scol,i,tau)].
  - 2-packed matmuls lhsT[128,96] @ hidden[128,128] -> PSUM [96,128] fp32,
    quantized to int8 and DMA'd to HBM in the natural [i,t,h] layout.
"""

import os
import sys
from concurrent.futures import ThreadPoolExecutor

sys.path.insert(0, "/opt/trn_rl_repo")

import numpy as np

import bass_rust
import concourse.bass as bass
import concourse.tile as tile
from concourse import bacc, mybir

F32 = mybir.dt.float32
F16 = mybir.dt.float16
I8 = mybir.dt.int8
ALU = mybir.AluOpType

B, N, T, H = 16, 48, 100, 128
NCORES = 8
NCHUNK = 2
BL = B // (NCORES * NCHUNK)   # batches per core per chunk (1)
CB = NCORES * BL              # batches per chunk (8)
TAU = T // 2                  # 50
TP = T + 2                    # out rows incl. 2 packed-scale rows
E = N * N                     # 2304 (full pair matrix incl. diag)
EPS = 1e-5
BIG = 60000.0                 # diag mask: 1/BIG ~ 1.7e-5 ~ 0
GI = 8                        # i's per PSUM transpose group


def _ap(t, offset, dims):
    """Manual access pattern on a tile handle's underlying tensor."""
    return bass_rust.AP(t.tensor, offset, [list(d) for d in dims])


def _dap(dram, offset, dims):
    """Manual access pattern on a DRAM tensor handle."""
    return bass_rust.AP(dram.ap().tensor, offset, [list(d) for d in dims])


def build_nc():
    nc = bacc.Bacc("TRN2", target_bir_lowering=False, debug=False)

    xt = nc.dram_tensor("xt", [2, 128, BL * N], F32, kind="ExternalInput")
    hidq = nc.dram_tensor("hidq", [BL, N, T, H], I8, kind="ExternalInput")
    hsc = nc.dram_tensor("hsc", [128, BL * N], F16, kind="ExternalInput")
    out_q = nc.dram_tensor("out_q", [BL, N, TP, H], I8, kind="ExternalOutput")

    row = (BIG * np.eye(N, dtype=np.float32)).astype(np.float16).reshape(1, E)
    bm = nc.inline_tensor(
        np.ascontiguousarray(np.repeat(row, 128, axis=0)), name="bm"
    )
    idm = np.zeros((128, TAU), dtype=np.float16)
    idm[0:TAU] = np.eye(TAU, dtype=np.float16)
    idm[64 : 64 + TAU] = np.eye(TAU, dtype=np.float16)
    ident = nc.inline_tensor(idm, name="ident")

    with tile.TileContext(nc) as tc:
        _emit(nc, tc, xt, hidq, hsc, bm, ident, out_q)
    nc.compile()
    return nc


def _emit(nc, tc, xt, hidq, hsc, bm, ident, out_q):
    FREE = BL * E             # free elems/partition for pair tiles
    LFREE = BL * 2 * N * TAU
    HF = BL * TAU * H

    with (
        tc.tile_pool(name="persist", bufs=1) as pp,
        tc.tile_pool(name="tp", bufs=3, space="PSUM") as tp_pool,
        tc.tile_pool(name="mm", bufs=4, space="PSUM") as mm_pool,
        tc.tile_pool(name="ot", bufs=6) as ot_pool,
        tc.tile_pool(name="rc", bufs=4) as rc_pool,
    ):
        xt_sb = pp.tile([128, 2 * BL * N], F32, tag="xt")
        hq_sb = pp.tile([128, HF], I8, tag="hq")
        hid_sb = pp.tile([128, HF], F16, tag="hid")
        bm_sb = pp.tile([128, E], F16, tag="bm")
        id_sb = pp.tile([128, TAU], F16, tag="id")
        hs_sb = pp.tile([128, BL * N], F16, tag="hs")
        dx = pp.tile([128, FREE], F32, tag="dx")
        dy = pp.tile([128, FREE], F32, tag="dy")
        dx2 = pp.tile([128, FREE], F32, tag="dx2")
        dy2 = pp.tile([128, FREE], F32, tag="dy2")
        adj16 = pp.tile([128, FREE], F16, tag="adj16")
        lhsT = pp.tile([128, LFREE], F16, tag="lhsT")
        osc_sb = pp.tile([96, BL * TAU], F16, tag="osc")
        tiny = pp.tile([96, 1], F32, tag="tiny")

        # ---- loads -------------------------------------------------------
        nc.sync.dma_start(xt_sb[:], xt.ap().rearrange("c p q -> p c q"))
        # hidden int8, native [b, j, t, h] -> rows s*64+j, free (b, tau, h)
        for s in range(2):
            for b in range(BL):
                src = _dap(
                    hidq, b * N * T * H + s * H, [[T * H, N], [2 * H, TAU], [1, H]]
                )
                dst = _ap(
                    hq_sb[:], (s * 64) * HF + b * (TAU * H),
                    [[HF, N], [H, TAU], [1, H]],
                )
                nc.sync.dma_start(dst, src)
        nc.sync.dma_start(bm_sb[:], bm.ap())
        nc.sync.dma_start(id_sb[:], ident.ap())
        nc.sync.dma_start(hs_sb[:], hsc.ap())
        nc.vector.memset(hid_sb[:], 0.0)
        nc.vector.memset(lhsT[:], 0.0)
        nc.vector.memset(tiny[:], 1e-12)
        # int8 -> fp16 (scale is folded into adj weights instead)
        for s in range(2):
            nc.scalar.copy(hid_sb[s * 64 : s * 64 + N, :], hq_sb[s * 64 : s * 64 + N, :])

        # ---- pairwise distance chain ------------------------------------
        # xt_sb free layout: (c, b, n); strides c:BL*N, b:N, n:1
        IH = N // 2               # 24 i's per chunk
        CH = IH * N               # free elems per chunk
        for b in range(BL):
            for ih in range(2):
                i0 = ih * IH
                off = b * E + i0 * N
                pv = lambda tl: _ap(tl[:], off, [[FREE, 128], [N, IH], [1, N]])
                fl = lambda tl: _ap(tl[:], off, [[FREE, 128], [1, CH]])
                cb = lambda c, vi: _ap(
                    xt_sb[:], c * (BL * N) + b * N + (i0 if vi else 0),
                    [[2 * BL * N, 128], [1, IH], [0, N]] if vi
                    else [[2 * BL * N, 128], [0, IH], [1, N]],
                )
                nc.vector.tensor_tensor(pv(dx), cb(0, True), cb(0, False), ALU.subtract)
                nc.vector.tensor_tensor(pv(dy), cb(1, True), cb(1, False), ALU.subtract)
                nc.scalar.square(fl(dx2), fl(dx))
                nc.scalar.square(fl(dy2), fl(dy))
                nc.gpsimd.tensor_tensor(fl(dx), fl(dx2), fl(dy2), ALU.add)
                nc.scalar.sqrt(fl(dy), fl(dx))
                bm_ap = _ap(bm_sb[:], i0 * N, [[E, 128], [1, CH]])
                nc.vector.scalar_tensor_tensor(
                    fl(dx2), fl(dy), EPS, bm_ap, ALU.add, ALU.add
                )
                nc.vector.reciprocal_approx_fast(out=fl(dy2), in_=fl(dx2))
                # adj16 = w * hscale[b, j]  (broadcast over i), f32*f16 -> f16
                hs_ap = _ap(hs_sb[:], b * N, [[BL * N, 128], [0, IH], [1, N]])
                nc.vector.tensor_tensor(pv(adj16), pv(dy2), hs_ap, ALU.mult)

                for s in range(2):
                    for gl in range(IH // GI):
                        g = i0 // GI + gl
                        pt = tp_pool.tile([48, GI * TAU], F16, tag="tp")
                        for ii in range(GI):
                            i = g * GI + ii
                            src = adj16[s * 64 : s * 64 + TAU,
                                        b * E + i * N : b * E + i * N + N]
                            nc.tensor.transpose(
                                pt[:, ii * TAU : (ii + 1) * TAU], src,
                                id_sb[s * 64 : s * 64 + TAU, :]
                            )
                        dst = _ap(
                            lhsT[:],
                            (s * 64) * LFREE + b * (2 * N * TAU) + s * (N * TAU)
                            + g * GI * TAU,
                            [[LFREE, 48], [TAU, GI], [1, TAU]],
                        )
                        csrc = _ap(pt[:], 0, [[GI * TAU, 48], [TAU, GI], [1, TAU]])
                        if g % 2 == 0:
                            nc.vector.tensor_copy(dst, csrc)
                        else:
                            nc.scalar.copy(dst, csrc)

        # ---- packed matmuls + int8 quantize + store ---------------------
        groups = [(tg * 4, min(4, TAU - tg * 4)) for tg in range((TAU + 3) // 4)]
        for b in range(BL):
            for t0, tlen in groups:
                mt = mm_pool.tile([96, 4 * H], F32, tag="mm")
                for k in range(tlen):
                    tau = t0 + k
                    w_ap = _ap(
                        lhsT[:], b * (2 * N * TAU) + tau, [[LFREE, 128], [TAU, 96]]
                    )
                    r_ap = _ap(
                        hid_sb[:], b * (TAU * H) + tau * H, [[HF, 128], [1, H]]
                    )
                    nc.tensor.matmul(
                        mt[:, k * H : (k + 1) * H], w_ap, r_ap,
                        start=True, stop=True,
                    )
                mt_view = _ap(mt[:], 0, [[4 * H, 96], [H, tlen], [1, H]])
                # per-(row, tau) amax over H -> osc (fp16, also the host scale)
                osc_slice = _ap(osc_sb[:], b * TAU + t0, [[BL * TAU, 96], [1, tlen]])
                nc.vector.reduce_max(
                    osc_slice, mt_view,
                    axis=mybir.AxisListType.X, apply_absolute_value=True,
                )
                rec = rc_pool.tile([96, 4], F32, tag="rc")
                rec_ap = _ap(rec[:], 0, [[4, 96], [1, tlen]])
                nc.vector.scalar_tensor_tensor(
                    rec_ap, osc_slice, 1.0 / 127.0,
                    _ap(tiny[:], 0, [[1, 96], [0, tlen]]), ALU.mult, ALU.add
                )
                nc.vector.reciprocal(out=rec_ap, in_=rec_ap)
                q8 = ot_pool.tile([96, 4 * H], I8, tag="ot")
                q8_view = _ap(q8[:], 0, [[4 * H, 96], [H, tlen], [1, H]])
                nc.vector.tensor_tensor(
                    q8_view, mt_view,
                    _ap(rec[:], 0, [[4, 96], [1, tlen], [0, H]]), ALU.mult
                )
                # scatter to native [b, i, t=2*tau+s, h]; partitions p=(s,i)
                for s in range(2):
                    dst = _dap(
                        out_q, b * N * TP * H + s * H + t0 * 2 * H,
                        [[TP * H, N], [2 * H, tlen], [1, H]],
                    )
                    qsrc = _ap(
                        q8[:], (s * N) * (4 * H), [[4 * H, N], [H, tlen], [1, H]]
                    )
                    nc.sync.dma_start(dst, qsrc)
        # pack fp16 amax scales into out rows T (s=0) / T+1 (s=1):
        # out_q[b, i, T+s, 0:2*TAU] <- bytes of osc_sb[s*48+i, b*TAU : (b+1)*TAU]
        out16 = out_q.bitcast(F16)          # [BL, N, TP, H//2] view
        H2 = H // 2
        for s in range(2):
            for b in range(BL):
                src = _ap(
                    osc_sb[:], (s * N) * (BL * TAU) + b * TAU,
                    [[BL * TAU, N], [1, TAU]],
                )
                dst = _dap(
                    out16, b * N * TP * H2 + (T + s) * H2,
                    [[TP * H2, N], [1, TAU]],
                )
                nc.sync.dma_start(dst, src)


# ----------------------------------------------------------------------------
# Host side
# ----------------------------------------------------------------------------

_RUN = None
_POOL = None
LAST_EXEC_NS = None


def _pool():
    global _POOL
    if _POOL is None:
        _POOL = ThreadPoolExecutor(max_workers=8)
    return _POOL


def _build_run():
    global _RUN
    if _RUN is not None:
        return _RUN
    import jax
    from jax.sharding import Mesh, PartitionSpec
    from jax.experimental.shard_map import shard_map
    from concourse.bass2jax import (
        _bass_exec_p,
        fast_dispatch_compile,
        install_neuronx_cc_hook,
        partition_id_tensor,
    )

    nc = build_nc()
    install_neuronx_cc_hook()

    partition_name = (
        nc.partition_id_tensor.name if nc.partition_id_tensor is not None else None
    )
    in_names, out_names, out_avals = [], [], []
    for alloc in nc.m.functions[0].allocations:
        if not isinstance(alloc, mybir.MemoryLocationSet):
            continue
        name = alloc.memorylocations[0].name
        if alloc.kind == "ExternalInput":
            if name != partition_name:
                in_names.append(name)
        elif alloc.kind == "ExternalOutput":
            out_names.append(name)
            out_avals.append(
                jax.core.ShapedArray(
                    tuple(alloc.tensor_shape), mybir.dt.np(alloc.dtype)
                )
            )
    bind_names = list(in_names) + ([partition_name] if partition_name else [])

    def _body(*args):
        operands = list(args)
        if partition_name is not None:
            operands.append(partition_id_tensor())
        outs = _bass_exec_p.bind(
            *operands,
            out_avals=tuple(out_avals),
            in_names=tuple(bind_names),
            out_names=tuple(out_names),
            lowering_input_output_aliases=(),
            sim_require_finite=True,
            sim_require_nnan=True,
            nc=nc,
        )
        return tuple(outs)

    devices = jax.devices()[:NCORES]
    mesh = Mesh(np.asarray(devices), ("core",))

    shapes = {}
    for alloc in nc.m.functions[0].allocations:
        if not isinstance(alloc, mybir.MemoryLocationSet):
            continue
        name = alloc.memorylocations[0].name
        if name in in_names:
            shp = list(alloc.tensor_shape)
            shp[0] *= NCORES
            shapes[name] = jax.ShapeDtypeStruct(
                tuple(shp), mybir.dt.np(alloc.dtype)
            )

    def _compile():
        jitted = jax.jit(
            shard_map(
                _body,
                mesh=mesh,
                in_specs=(PartitionSpec("core"),) * len(in_names),
                out_specs=(PartitionSpec("core"),) * len(out_names),
                check_rep=False,
            )
        )
        return jitted.lower(*[shapes[n] for n in in_names]).compile()

    sharded = fast_dispatch_compile(_compile)
    _RUN = (sharded, in_names, out_names)
    return _RUN


def _quant_chunk(locs_c, hidden_c):
    """locs_c [CB,N,T,2] f32, hidden_c [CB,N,T,H] f32 -> device input map."""
    # int8-quantize hidden with per-(b,n,t) scale amax/127 (threaded over batch)
    amax = np.empty((CB, N, T), np.float32)
    hidq = np.empty((CB, N, T, H), np.int8)

    def _qb(b):
        a = np.abs(hidden_c[b]).max(-1)
        amax[b] = a
        tmp = hidden_c[b] * (127.0 / (a + 1e-30))[..., None]
        np.rint(tmp, out=tmp)
        hidq[b] = tmp.astype(np.int8)

    list(_pool().map(_qb, range(CB)))

    # coords: per-core [2(coord), 128(s*64+tau), BL*N]
    l6 = locs_c.reshape(NCORES, BL, N, TAU, 2, 2)       # c b n tau s coord
    xtz = np.zeros((NCORES, 2, 2, 64, BL, N), np.float32)
    xtz[:, :, :, :TAU] = l6.transpose(0, 5, 4, 3, 1, 2)
    # filler rows: spread points (x=n, y=0) so junk weights stay finite
    xtz[:, 0, :, TAU:] = np.arange(N, dtype=np.float32)
    xt = xtz.reshape(CB * 2 // BL, 128, BL * N)

    # hidden scales for the device: rows (s,tau), cols (b,j), value amax/127
    s6 = (amax * (1.0 / 127.0)).astype(np.float16)
    s6 = s6.reshape(NCORES, BL, N, TAU, 2)              # c b j tau s
    hscz = np.zeros((NCORES, 2, 64, BL, N), np.float16)
    hscz[:, :, :TAU] = s6.transpose(0, 4, 3, 1, 2)
    hsc = hscz.reshape(NCORES * 128, BL * N)
    return {"xt": xt, "hidq": hidq, "hsc": hsc}


def _dequant_chunk(buf, out_c):
    """buf [CB,N,TP,H] int8 (incl. packed scales) -> out_c [CB,N,T,H] f32."""
    scb = np.ascontiguousarray(buf[:, :, T : T + 2, : TAU * 2])
    sc = scb.view(np.float16).astype(np.float32)        # [CB, N, 2(s), TAU]
    sc = sc.transpose(0, 1, 3, 2).reshape(CB, N, T) * (1.0 / 127.0)

    def _db(b):
        out_c[b] = buf[b, :, :T, :]
        out_c[b] *= sc[b][..., None]

    list(_pool().map(_db, range(CB)))


def kernel(locs, hidden, rel_rec=None, rel_send=None):
    locs = np.asarray(locs, dtype=np.float32)
    hidden = np.asarray(hidden, dtype=np.float32)
    sharded, in_names, out_names = _build_run()

    outs = []
    for k in range(NCHUNK):
        ins = _quant_chunk(
            locs[k * CB : (k + 1) * CB], hidden[k * CB : (k + 1) * CB]
        )
        outs.append(sharded(*[ins[n] for n in in_names]))

    out = np.empty((B, N, T, H), np.float32)
    for k in range(NCHUNK):
        buf = np.asarray(outs[k][0]).reshape(CB, N, TP, H)
        _dequant_chunk(buf, out[k * CB : (k + 1) * CB])
    return out


if __name__ == "__main__":
    # smoke test: two different random datasets against a local numpy reference
    rng = np.random.default_rng(0)
    for trial in range(2):
        locs = rng.standard_normal((B, N, T, 2), dtype=np.float32)
        hidden = rng.standard_normal((B, N, T, H), dtype=np.float32)
        got = kernel(locs, hidden)
        x = locs[..., 0]
        y = locs[..., 1]
        d = np.sqrt(
            (x[:, :, None] - x[:, None]) ** 2 + (y[:, :, None] - y[:, None]) ** 2
        )
        w = 1.0 / (d + EPS) * (1.0 - np.eye(N)[None, :, :, None])
        want = np.einsum("bijt,bjth->bith", w.astype(np.float32), hidden)
        err = np.linalg.norm(got - want) / np.linalg.norm(want)
        print(f"trial {trial} rel err vs numpy: {err}")


# revision 18
# speedup vs baseline: 1.2747x; 1.2747x over previous
"""Trainium2 Bass kernel for nn_HardwiredAttention (NRI-style GNN message passing).

Math (derived from the reference):
  adj[b,t,i,j] = 1/(||locs[b,i,t]-locs[b,j,t]|| + eps) for i!=j, ~0 on diag
  out[b,:,t,:] = adj[b,t] @ hidden[b,:,t,:]          ([48,48] @ [48,128] per (b,t))

Distribution: data-parallel over batch, 8 cores; the batch is processed in
NCHUNK pipelined chunks of 8 (one batch per core per call) so host quant /
dequant and device exec overlap the host<->device link transfers.

The end-to-end call is dominated by the host<->device link (~45 MB/s tunnel),
so the kernel minimizes bytes moved:
  - hidden is int8-quantized host-side with a per-(b,n,t) fp16 scale; the
    scale is folded into the adjacency weights on device (w'_ij = w_ij * s_j),
    so the device only does an int8->fp16 convert on the hidden payload.
  - the output is int8-quantized on device (per-(b,i,t) amax over H via
    reduce_max(abs), round-to-nearest saturating convert) and dequantized
    host-side. The fp16 amax scales are packed into two spare T-rows of the
    same int8 output tensor, so one fetch returns everything.
  - the diag mask / PE-transpose identity are baked into the NEFF as Const
    tensors (no per-call transfer), and no zero output buffers are donated
    (the kernel writes every output element).
  - the jitted shard_map callable is built once and cached.

Per-core device layout (same skeleton as the fp16 baseline):
  - elementwise pipeline in partitions p=(s,tau), t=2*tau+s (100 partitions):
    dx/dy from a tiny [100,(c,n)] coords tile via stride-0 broadcast APs,
    squares on ACT, d2-add on GPSIMD, sqrt on ACT, (d+eps)+BIGMASK via
    scalar_tensor_tensor, reciprocal_approx_fast, then *hscale -> fp16 adj.
  - PE transposes [50(tau),48(j)] -> [48(j),50(tau)] per (i,s) build a
    block-diagonal fp16 lhsT [128=(s,j), (scol,i,tau)].
  - 2-packed matmuls lhsT[128,96] @ hidden[128,128] -> PSUM [96,128] fp32,
    quantized to int8 and DMA'd to HBM in the natural [i,t,h] layout.
"""

import os
import sys
from concurrent.futures import ThreadPoolExecutor

sys.path.insert(0, "/opt/trn_rl_repo")

import numpy as np

import bass_rust
import concourse.bass as bass
import concourse.tile as tile
from concourse import bacc, mybir

F32 = mybir.dt.float32
F16 = mybir.dt.float16
I8 = mybir.dt.int8
ALU = mybir.AluOpType

B, N, T, H = 16, 48, 100, 128
NCORES = 8
NCHUNK = 2
BL = B // (NCORES * NCHUNK)   # batches per core per chunk (1)
CB = NCORES * BL              # batches per chunk (8)
TAU = T // 2                  # 50
TP = T + 2                    # out rows incl. 2 packed-scale rows
E = N * N                     # 2304 (full pair matrix incl. diag)
EPS = 1e-5
BIG = 60000.0                 # diag mask: 1/BIG ~ 1.7e-5 ~ 0
GI = 8                        # i's per PSUM transpose group


def _ap(t, offset, dims):
    """Manual access pattern on a tile handle's underlying tensor."""
    return bass_rust.AP(t.tensor, offset, [list(d) for d in dims])


def _dap(dram, offset, dims):
    """Manual access pattern on a DRAM tensor handle."""
    return bass_rust.AP(dram.ap().tensor, offset, [list(d) for d in dims])


def build_nc():
    nc = bacc.Bacc("TRN2", target_bir_lowering=False, debug=False)

    xt = nc.dram_tensor("xt", [2, 128, BL * N], F32, kind="ExternalInput")
    hidq = nc.dram_tensor("hidq", [BL, N, T, H], I8, kind="ExternalInput")
    hsc = nc.dram_tensor("hsc", [128, BL * N], F16, kind="ExternalInput")
    out_q = nc.dram_tensor("out_q", [BL, N, TP, H], I8, kind="ExternalOutput")

    row = (BIG * np.eye(N, dtype=np.float32)).astype(np.float16).reshape(1, E)
    bm = nc.inline_tensor(
        np.ascontiguousarray(np.repeat(row, 128, axis=0)), name="bm"
    )
    idm = np.zeros((128, TAU), dtype=np.float16)
    idm[0:TAU] = np.eye(TAU, dtype=np.float16)
    idm[64 : 64 + TAU] = np.eye(TAU, dtype=np.float16)
    ident = nc.inline_tensor(idm, name="ident")

    with tile.TileContext(nc) as tc:
        _emit(nc, tc, xt, hidq, hsc, bm, ident, out_q)
    nc.compile()
    return nc


def _emit(nc, tc, xt, hidq, hsc, bm, ident, out_q):
    FREE = BL * E             # free elems/partition for pair tiles
    LFREE = BL * 2 * N * TAU
    HF = BL * TAU * H

    with (
        tc.tile_pool(name="persist", bufs=1) as pp,
        tc.tile_pool(name="tp", bufs=3, space="PSUM") as tp_pool,
        tc.tile_pool(name="mm", bufs=4, space="PSUM") as mm_pool,
        tc.tile_pool(name="ot", bufs=6) as ot_pool,
        tc.tile_pool(name="rc", bufs=4) as rc_pool,
    ):
        xt_sb = pp.tile([128, 2 * BL * N], F32, tag="xt")
        hq_sb = pp.tile([128, HF], I8, tag="hq")
        hid_sb = pp.tile([128, HF], F16, tag="hid")
        bm_sb = pp.tile([128, E], F16, tag="bm")
        id_sb = pp.tile([128, TAU], F16, tag="id")
        hs_sb = pp.tile([128, BL * N], F16, tag="hs")
        dx = pp.tile([128, FREE], F32, tag="dx")
        dy = pp.tile([128, FREE], F32, tag="dy")
        dx2 = pp.tile([128, FREE], F32, tag="dx2")
        dy2 = pp.tile([128, FREE], F32, tag="dy2")
        adj16 = pp.tile([128, FREE], F16, tag="adj16")
        lhsT = pp.tile([128, LFREE], F16, tag="lhsT")
        osc_sb = pp.tile([96, BL * TAU], F16, tag="osc")
        tiny = pp.tile([96, 1], F32, tag="tiny")

        # ---- loads -------------------------------------------------------
        nc.sync.dma_start(xt_sb[:], xt.ap().rearrange("c p q -> p c q"))
        # hidden int8, native [b, j, t, h] -> rows s*64+j, free (b, tau, h)
        for s in range(2):
            for b in range(BL):
                src = _dap(
                    hidq, b * N * T * H + s * H, [[T * H, N], [2 * H, TAU], [1, H]]
                )
                dst = _ap(
                    hq_sb[:], (s * 64) * HF + b * (TAU * H),
                    [[HF, N], [H, TAU], [1, H]],
                )
                nc.sync.dma_start(dst, src)
        nc.sync.dma_start(bm_sb[:], bm.ap())
        nc.sync.dma_start(id_sb[:], ident.ap())
        nc.sync.dma_start(hs_sb[:], hsc.ap())
        nc.vector.memset(hid_sb[:], 0.0)
        nc.vector.memset(lhsT[:], 0.0)
        nc.vector.memset(tiny[:], 1e-12)
        # int8 -> fp16 (scale is folded into adj weights instead)
        for s in range(2):
            nc.scalar.copy(hid_sb[s * 64 : s * 64 + N, :], hq_sb[s * 64 : s * 64 + N, :])

        # ---- pairwise distance chain ------------------------------------
        # xt_sb free layout: (c, b, n); strides c:BL*N, b:N, n:1
        IH = N // 2               # 24 i's per chunk
        CH = IH * N               # free elems per chunk
        for b in range(BL):
            for ih in range(2):
                i0 = ih * IH
                off = b * E + i0 * N
                pv = lambda tl: _ap(tl[:], off, [[FREE, 128], [N, IH], [1, N]])
                fl = lambda tl: _ap(tl[:], off, [[FREE, 128], [1, CH]])
                cb = lambda c, vi: _ap(
                    xt_sb[:], c * (BL * N) + b * N + (i0 if vi else 0),
                    [[2 * BL * N, 128], [1, IH], [0, N]] if vi
                    else [[2 * BL * N, 128], [0, IH], [1, N]],
                )
                nc.vector.tensor_tensor(pv(dx), cb(0, True), cb(0, False), ALU.subtract)
                nc.vector.tensor_tensor(pv(dy), cb(1, True), cb(1, False), ALU.subtract)
                nc.scalar.square(fl(dx2), fl(dx))
                nc.scalar.square(fl(dy2), fl(dy))
                nc.gpsimd.tensor_tensor(fl(dx), fl(dx2), fl(dy2), ALU.add)
                nc.scalar.sqrt(fl(dy), fl(dx))
                bm_ap = _ap(bm_sb[:], i0 * N, [[E, 128], [1, CH]])
                nc.vector.scalar_tensor_tensor(
                    fl(dx2), fl(dy), EPS, bm_ap, ALU.add, ALU.add
                )
                nc.vector.reciprocal_approx_fast(out=fl(dy2), in_=fl(dx2))
                # adj16 = w * hscale[b, j]  (broadcast over i), f32*f16 -> f16
                hs_ap = _ap(hs_sb[:], b * N, [[BL * N, 128], [0, IH], [1, N]])
                nc.vector.tensor_tensor(pv(adj16), pv(dy2), hs_ap, ALU.mult)

                for s in range(2):
                    for gl in range(IH // GI):
                        g = i0 // GI + gl
                        pt = tp_pool.tile([48, GI * TAU], F16, tag="tp")
                        for ii in range(GI):
                            i = g * GI + ii
                            src = adj16[s * 64 : s * 64 + TAU,
                                        b * E + i * N : b * E + i * N + N]
                            nc.tensor.transpose(
                                pt[:, ii * TAU : (ii + 1) * TAU], src,
                                id_sb[s * 64 : s * 64 + TAU, :]
                            )
                        dst = _ap(
                            lhsT[:],
                            (s * 64) * LFREE + b * (2 * N * TAU) + s * (N * TAU)
                            + g * GI * TAU,
                            [[LFREE, 48], [TAU, GI], [1, TAU]],
                        )
                        csrc = _ap(pt[:], 0, [[GI * TAU, 48], [TAU, GI], [1, TAU]])
                        if g % 2 == 0:
                            nc.vector.tensor_copy(dst, csrc)
                        else:
                            nc.scalar.copy(dst, csrc)

        # ---- packed matmuls + int8 quantize + store ---------------------
        groups = [(tg * 4, min(4, TAU - tg * 4)) for tg in range((TAU + 3) // 4)]
        for b in range(BL):
            for t0, tlen in groups:
                mt = mm_pool.tile([96, 4 * H], F32, tag="mm")
                for k in range(tlen):
                    tau = t0 + k
                    w_ap = _ap(
                        lhsT[:], b * (2 * N * TAU) + tau, [[LFREE, 128], [TAU, 96]]
                    )
                    r_ap = _ap(
                        hid_sb[:], b * (TAU * H) + tau * H, [[HF, 128], [1, H]]
                    )
                    nc.tensor.matmul(
                        mt[:, k * H : (k + 1) * H], w_ap, r_ap,
                        start=True, stop=True,
                    )
                mt_view = _ap(mt[:], 0, [[4 * H, 96], [H, tlen], [1, H]])
                # per-(row, tau) amax over H -> osc (fp16, also the host scale)
                osc_slice = _ap(osc_sb[:], b * TAU + t0, [[BL * TAU, 96], [1, tlen]])
                nc.vector.reduce_max(
                    osc_slice, mt_view,
                    axis=mybir.AxisListType.X, apply_absolute_value=True,
                )
                rec = rc_pool.tile([96, 4], F32, tag="rc")
                rec_ap = _ap(rec[:], 0, [[4, 96], [1, tlen]])
                nc.vector.scalar_tensor_tensor(
                    rec_ap, osc_slice, 1.0 / 127.0,
                    _ap(tiny[:], 0, [[1, 96], [0, tlen]]), ALU.mult, ALU.add
                )
                nc.vector.reciprocal(out=rec_ap, in_=rec_ap)
                q8 = ot_pool.tile([96, 4 * H], I8, tag="ot")
                q8_view = _ap(q8[:], 0, [[4 * H, 96], [H, tlen], [1, H]])
                nc.vector.tensor_tensor(
                    q8_view, mt_view,
                    _ap(rec[:], 0, [[4, 96], [1, tlen], [0, H]]), ALU.mult
                )
                # scatter to native [b, i, t=2*tau+s, h]; partitions p=(s,i)
                for s in range(2):
                    dst = _dap(
                        out_q, b * N * TP * H + s * H + t0 * 2 * H,
                        [[TP * H, N], [2 * H, tlen], [1, H]],
                    )
                    qsrc = _ap(
                        q8[:], (s * N) * (4 * H), [[4 * H, N], [H, tlen], [1, H]]
                    )
                    nc.sync.dma_start(dst, qsrc)
        # pack fp16 amax scales into out rows T (s=0) / T+1 (s=1):
        # out_q[b, i, T+s, 0:2*TAU] <- bytes of osc_sb[s*48+i, b*TAU : (b+1)*TAU]
        out16 = out_q.bitcast(F16)          # [BL, N, TP, H//2] view
        H2 = H // 2
        for s in range(2):
            for b in range(BL):
                src = _ap(
                    osc_sb[:], (s * N) * (BL * TAU) + b * TAU,
                    [[BL * TAU, N], [1, TAU]],
                )
                dst = _dap(
                    out16, b * N * TP * H2 + (T + s) * H2,
                    [[TP * H2, N], [1, TAU]],
                )
                nc.sync.dma_start(dst, src)


# ----------------------------------------------------------------------------
# Host side
# ----------------------------------------------------------------------------

_RUN = None
_POOL = None
_FPOOL = None
LAST_EXEC_NS = None


def _pool():
    global _POOL
    if _POOL is None:
        _POOL = ThreadPoolExecutor(max_workers=8)
    return _POOL


def _fetch_pool():
    global _FPOOL
    if _FPOOL is None:
        _FPOOL = ThreadPoolExecutor(max_workers=NCHUNK)
    return _FPOOL


_SPOOL = None


def _shard_pool():
    global _SPOOL
    if _SPOOL is None:
        _SPOOL = ThreadPoolExecutor(max_workers=2 * NCORES)
    return _SPOOL


def _build_run():
    global _RUN
    if _RUN is not None:
        return _RUN
    import jax
    from jax.sharding import Mesh, PartitionSpec
    from jax.experimental.shard_map import shard_map
    from concourse.bass2jax import (
        _bass_exec_p,
        fast_dispatch_compile,
        install_neuronx_cc_hook,
        partition_id_tensor,
    )

    nc = build_nc()
    install_neuronx_cc_hook()

    partition_name = (
        nc.partition_id_tensor.name if nc.partition_id_tensor is not None else None
    )
    in_names, out_names, out_avals = [], [], []
    for alloc in nc.m.functions[0].allocations:
        if not isinstance(alloc, mybir.MemoryLocationSet):
            continue
        name = alloc.memorylocations[0].name
        if alloc.kind == "ExternalInput":
            if name != partition_name:
                in_names.append(name)
        elif alloc.kind == "ExternalOutput":
            out_names.append(name)
            out_avals.append(
                jax.core.ShapedArray(
                    tuple(alloc.tensor_shape), mybir.dt.np(alloc.dtype)
                )
            )
    bind_names = list(in_names) + ([partition_name] if partition_name else [])

    def _body(*args):
        operands = list(args)
        if partition_name is not None:
            operands.append(partition_id_tensor())
        outs = _bass_exec_p.bind(
            *operands,
            out_avals=tuple(out_avals),
            in_names=tuple(bind_names),
            out_names=tuple(out_names),
            lowering_input_output_aliases=(),
            sim_require_finite=True,
            sim_require_nnan=True,
            nc=nc,
        )
        return tuple(outs)

    devices = jax.devices()[:NCORES]
    mesh = Mesh(np.asarray(devices), ("core",))

    shapes = {}
    for alloc in nc.m.functions[0].allocations:
        if not isinstance(alloc, mybir.MemoryLocationSet):
            continue
        name = alloc.memorylocations[0].name
        if name in in_names:
            shp = list(alloc.tensor_shape)
            shp[0] *= NCORES
            shapes[name] = jax.ShapeDtypeStruct(
                tuple(shp), mybir.dt.np(alloc.dtype)
            )

    def _compile():
        jitted = jax.jit(
            shard_map(
                _body,
                mesh=mesh,
                in_specs=(PartitionSpec("core"),) * len(in_names),
                out_specs=(PartitionSpec("core"),) * len(out_names),
                check_rep=False,
            )
        )
        return jitted.lower(*[shapes[n] for n in in_names]).compile()

    sharded = fast_dispatch_compile(_compile)
    _RUN = (sharded, in_names, out_names)
    return _RUN


def _quant_chunk(locs_c, hidden_c):
    """locs_c [CB,N,T,2] f32, hidden_c [CB,N,T,H] f32 -> device input map."""
    # int8-quantize hidden with per-(b,n,t) scale amax/127 (threaded over batch)
    amax = np.empty((CB, N, T), np.float32)
    hidq = np.empty((CB, N, T, H), np.int8)

    def _qb(b):
        a = np.abs(hidden_c[b]).max(-1)
        amax[b] = a
        tmp = hidden_c[b] * (127.0 / (a + 1e-30))[..., None]
        np.rint(tmp, out=tmp)
        hidq[b] = tmp.astype(np.int8)

    list(_pool().map(_qb, range(CB)))

    # coords: per-core [2(coord), 128(s*64+tau), BL*N]
    l6 = locs_c.reshape(NCORES, BL, N, TAU, 2, 2)       # c b n tau s coord
    xtz = np.zeros((NCORES, 2, 2, 64, BL, N), np.float32)
    xtz[:, :, :, :TAU] = l6.transpose(0, 5, 4, 3, 1, 2)
    # filler rows: spread points (x=n, y=0) so junk weights stay finite
    xtz[:, 0, :, TAU:] = np.arange(N, dtype=np.float32)
    xt = xtz.reshape(CB * 2 // BL, 128, BL * N)

    # hidden scales for the device: rows (s,tau), cols (b,j), value amax/127
    s6 = (amax * (1.0 / 127.0)).astype(np.float16)
    s6 = s6.reshape(NCORES, BL, N, TAU, 2)              # c b j tau s
    hscz = np.zeros((NCORES, 2, 64, BL, N), np.float16)
    hscz[:, :, :TAU] = s6.transpose(0, 4, 3, 1, 2)
    hsc = hscz.reshape(NCORES * 128, BL * N)
    return {"xt": xt, "hidq": hidq, "hsc": hsc}


def _dequant_chunk(buf, out_c):
    """buf [CB,N,TP,H] int8 (incl. packed scales) -> out_c [CB,N,T,H] f32."""
    scb = np.ascontiguousarray(buf[:, :, T : T + 2, : TAU * 2])
    sc = scb.view(np.float16).astype(np.float32)        # [CB, N, 2(s), TAU]
    sc = sc.transpose(0, 1, 3, 2).reshape(CB, N, T) * (1.0 / 127.0)

    def _db(b):
        out_c[b] = buf[b, :, :T, :]
        out_c[b] *= sc[b][..., None]

    list(_pool().map(_db, range(CB)))


def kernel(locs, hidden, rel_rec=None, rel_send=None):
    locs = np.asarray(locs, dtype=np.float32)
    hidden = np.asarray(hidden, dtype=np.float32)
    sharded, in_names, out_names = _build_run()

    outs = []
    for k in range(NCHUNK):
        ins = _quant_chunk(
            locs[k * CB : (k + 1) * CB], hidden[k * CB : (k + 1) * CB]
        )
        outs.append(sharded(*[ins[n] for n in in_names]))

    out = np.empty((B, N, T, H), np.float32)

    def _fetch_deq(k):
        # fetch the chunk's shards concurrently; dequant each batch as it lands
        futs = [
            (s.index[0].start, _shard_pool().submit(np.asarray, s.data))
            for s in outs[k][0].addressable_shards
        ]
        for row, fut in futs:
            buf = fut.result().reshape(BL * N, TP, H)   # one batch (BL=1)
            scb = np.ascontiguousarray(buf[:, T : T + 2, : TAU * 2])
            sc = scb.view(np.float16).astype(np.float32)    # [N, 2(s), TAU]
            sc = sc.transpose(0, 2, 1).reshape(N, T) * (1.0 / 127.0)
            o = out[k * CB + row]
            o[:] = buf[:, :T, :]
            o *= sc[..., None]

    list(_fetch_pool().map(_fetch_deq, range(NCHUNK)))
    return out


if __name__ == "__main__":
    # smoke test: two different random datasets against a local numpy reference
    rng = np.random.default_rng(0)
    for trial in range(2):
        locs = rng.standard_normal((B, N, T, 2), dtype=np.float32)
        hidden = rng.standard_normal((B, N, T, H), dtype=np.float32)
        got = kernel(locs, hidden)
        x = locs[..., 0]
        y = locs[..., 1]
        d = np.sqrt(
            (x[:, :, None] - x[:, None]) ** 2 + (y[:, :, None] - y[:, None]) ** 2
        )
        w = 1.0 / (d + EPS) * (1.0 - np.eye(N)[None, :, :, None])
        want = np.einsum("bijt,bjth->bith", w.astype(np.float32), hidden)
        err = np.linalg.norm(got - want) / np.linalg.norm(want)
        print(f"trial {trial} rel err vs numpy: {err}")


# revision 19
# speedup vs baseline: 1.3086x; 1.0266x over previous
"""Trainium2 Bass kernel for nn_HardwiredAttention (NRI-style GNN message passing).

Math (derived from the reference):
  adj[b,t,i,j] = 1/(||locs[b,i,t]-locs[b,j,t]|| + eps) for i!=j, ~0 on diag
  out[b,:,t,:] = adj[b,t] @ hidden[b,:,t,:]          ([48,48] @ [48,128] per (b,t))

Distribution: data-parallel over batch, 8 cores; the batch is processed in
NCHUNK pipelined chunks of 8 (one batch per core per call) so host quant /
dequant and device exec overlap the host<->device link transfers.

The end-to-end call is dominated by the host<->device link (~45 MB/s tunnel),
so the kernel minimizes bytes moved:
  - hidden is int8-quantized host-side with a per-(b,n,t) fp16 scale; the
    scale is folded into the adjacency weights on device (w'_ij = w_ij * s_j),
    so the device only does an int8->fp16 convert on the hidden payload.
  - the output is int8-quantized on device (per-(b,i,t) amax over H via
    reduce_max(abs), round-to-nearest saturating convert) and dequantized
    host-side. The fp16 amax scales are packed into two spare T-rows of the
    same int8 output tensor, so one fetch returns everything.
  - the diag mask / PE-transpose identity are baked into the NEFF as Const
    tensors (no per-call transfer), and no zero output buffers are donated
    (the kernel writes every output element).
  - the jitted shard_map callable is built once and cached.

Per-core device layout (same skeleton as the fp16 baseline):
  - elementwise pipeline in partitions p=(s,tau), t=2*tau+s (100 partitions):
    dx/dy from a tiny [100,(c,n)] coords tile via stride-0 broadcast APs,
    squares on ACT, d2-add on GPSIMD, sqrt on ACT, (d+eps)+BIGMASK via
    scalar_tensor_tensor, reciprocal_approx_fast, then *hscale -> fp16 adj.
  - PE transposes [50(tau),48(j)] -> [48(j),50(tau)] per (i,s) build a
    block-diagonal fp16 lhsT [128=(s,j), (scol,i,tau)].
  - 2-packed matmuls lhsT[128,96] @ hidden[128,128] -> PSUM [96,128] fp32,
    quantized to int8 and DMA'd to HBM in the natural [i,t,h] layout.
"""

import os
import sys
from concurrent.futures import ThreadPoolExecutor

sys.path.insert(0, "/opt/trn_rl_repo")

import numpy as np

import bass_rust
import concourse.bass as bass
import concourse.tile as tile
from concourse import bacc, mybir

F32 = mybir.dt.float32
F16 = mybir.dt.float16
I8 = mybir.dt.int8
ALU = mybir.AluOpType

B, N, T, H = 16, 48, 100, 128
NCORES = 8
NCHUNK = 2
BL = B // (NCORES * NCHUNK)   # batches per core per chunk (1)
CB = NCORES * BL              # batches per chunk (8)
TAU = T // 2                  # 50
TP = T + 2                    # out rows incl. 2 packed-scale rows
E = N * N                     # 2304 (full pair matrix incl. diag)
EPS = 1e-5
BIG = 60000.0                 # diag mask: 1/BIG ~ 1.7e-5 ~ 0
GI = 8                        # i's per PSUM transpose group


def _ap(t, offset, dims):
    """Manual access pattern on a tile handle's underlying tensor."""
    return bass_rust.AP(t.tensor, offset, [list(d) for d in dims])


def _dap(dram, offset, dims):
    """Manual access pattern on a DRAM tensor handle."""
    return bass_rust.AP(dram.ap().tensor, offset, [list(d) for d in dims])


def build_nc():
    nc = bacc.Bacc("TRN2", target_bir_lowering=False, debug=False)

    xt = nc.dram_tensor("xt", [2, 128, BL * N], F32, kind="ExternalInput")
    hidq = nc.dram_tensor("hidq", [BL, N, T, H], I8, kind="ExternalInput")
    hsc = nc.dram_tensor("hsc", [128, BL * N], F16, kind="ExternalInput")
    out_q = nc.dram_tensor("out_q", [BL, N, TP, H], I8, kind="ExternalOutput")

    row = (BIG * np.eye(N, dtype=np.float32)).astype(np.float16).reshape(1, E)
    bm = nc.inline_tensor(
        np.ascontiguousarray(np.repeat(row, 128, axis=0)), name="bm"
    )
    idm = np.zeros((128, TAU), dtype=np.float16)
    idm[0:TAU] = np.eye(TAU, dtype=np.float16)
    idm[64 : 64 + TAU] = np.eye(TAU, dtype=np.float16)
    ident = nc.inline_tensor(idm, name="ident")

    with tile.TileContext(nc) as tc:
        _emit(nc, tc, xt, hidq, hsc, bm, ident, out_q)
    nc.compile()
    return nc


def _emit(nc, tc, xt, hidq, hsc, bm, ident, out_q):
    FREE = BL * E             # free elems/partition for pair tiles
    LFREE = BL * 2 * N * TAU
    HF = BL * TAU * H

    with (
        tc.tile_pool(name="persist", bufs=1) as pp,
        tc.tile_pool(name="tp", bufs=3, space="PSUM") as tp_pool,
        tc.tile_pool(name="mm", bufs=4, space="PSUM") as mm_pool,
        tc.tile_pool(name="ot", bufs=6) as ot_pool,
        tc.tile_pool(name="rc", bufs=4) as rc_pool,
    ):
        xt_sb = pp.tile([128, 2 * BL * N], F32, tag="xt")
        hq_sb = pp.tile([128, HF], I8, tag="hq")
        hid_sb = pp.tile([128, HF], F16, tag="hid")
        bm_sb = pp.tile([128, E], F16, tag="bm")
        id_sb = pp.tile([128, TAU], F16, tag="id")
        hs_sb = pp.tile([128, BL * N], F16, tag="hs")
        dx = pp.tile([128, FREE], F32, tag="dx")
        dy = pp.tile([128, FREE], F32, tag="dy")
        dx2 = pp.tile([128, FREE], F32, tag="dx2")
        dy2 = pp.tile([128, FREE], F32, tag="dy2")
        adj16 = pp.tile([128, FREE], F16, tag="adj16")
        lhsT = pp.tile([128, LFREE], F16, tag="lhsT")
        osc_sb = pp.tile([96, BL * TAU], F16, tag="osc")
        tiny = pp.tile([96, 1], F32, tag="tiny")

        # ---- loads -------------------------------------------------------
        nc.sync.dma_start(xt_sb[:], xt.ap().rearrange("c p q -> p c q"))
        # hidden int8, native [b, j, t, h] -> rows s*64+j, free (b, tau, h)
        for s in range(2):
            for b in range(BL):
                src = _dap(
                    hidq, b * N * T * H + s * H, [[T * H, N], [2 * H, TAU], [1, H]]
                )
                dst = _ap(
                    hq_sb[:], (s * 64) * HF + b * (TAU * H),
                    [[HF, N], [H, TAU], [1, H]],
                )
                nc.sync.dma_start(dst, src)
        nc.sync.dma_start(bm_sb[:], bm.ap())
        nc.sync.dma_start(id_sb[:], ident.ap())
        nc.sync.dma_start(hs_sb[:], hsc.ap())
        nc.vector.memset(hid_sb[:], 0.0)
        nc.vector.memset(lhsT[:], 0.0)
        nc.vector.memset(tiny[:], 1e-12)
        # int8 -> fp16 (scale is folded into adj weights instead)
        for s in range(2):
            nc.scalar.copy(hid_sb[s * 64 : s * 64 + N, :], hq_sb[s * 64 : s * 64 + N, :])

        # ---- pairwise distance chain ------------------------------------
        # xt_sb free layout: (c, b, n); strides c:BL*N, b:N, n:1
        IH = N // 2               # 24 i's per chunk
        CH = IH * N               # free elems per chunk
        for b in range(BL):
            for ih in range(2):
                i0 = ih * IH
                off = b * E + i0 * N
                pv = lambda tl: _ap(tl[:], off, [[FREE, 128], [N, IH], [1, N]])
                fl = lambda tl: _ap(tl[:], off, [[FREE, 128], [1, CH]])
                cb = lambda c, vi: _ap(
                    xt_sb[:], c * (BL * N) + b * N + (i0 if vi else 0),
                    [[2 * BL * N, 128], [1, IH], [0, N]] if vi
                    else [[2 * BL * N, 128], [0, IH], [1, N]],
                )
                nc.vector.tensor_tensor(pv(dx), cb(0, True), cb(0, False), ALU.subtract)
                nc.vector.tensor_tensor(pv(dy), cb(1, True), cb(1, False), ALU.subtract)
                nc.scalar.square(fl(dx2), fl(dx))
                nc.scalar.square(fl(dy2), fl(dy))
                nc.gpsimd.tensor_tensor(fl(dx), fl(dx2), fl(dy2), ALU.add)
                nc.scalar.sqrt(fl(dy), fl(dx))
                bm_ap = _ap(bm_sb[:], i0 * N, [[E, 128], [1, CH]])
                nc.vector.scalar_tensor_tensor(
                    fl(dx2), fl(dy), EPS, bm_ap, ALU.add, ALU.add
                )
                nc.vector.reciprocal_approx_fast(out=fl(dy2), in_=fl(dx2))
                # adj16 = w * hscale[b, j]  (broadcast over i), f32*f16 -> f16
                hs_ap = _ap(hs_sb[:], b * N, [[BL * N, 128], [0, IH], [1, N]])
                nc.vector.tensor_tensor(pv(adj16), pv(dy2), hs_ap, ALU.mult)

                for s in range(2):
                    for gl in range(IH // GI):
                        g = i0 // GI + gl
                        pt = tp_pool.tile([48, GI * TAU], F16, tag="tp")
                        for ii in range(GI):
                            i = g * GI + ii
                            src = adj16[s * 64 : s * 64 + TAU,
                                        b * E + i * N : b * E + i * N + N]
                            nc.tensor.transpose(
                                pt[:, ii * TAU : (ii + 1) * TAU], src,
                                id_sb[s * 64 : s * 64 + TAU, :]
                            )
                        dst = _ap(
                            lhsT[:],
                            (s * 64) * LFREE + b * (2 * N * TAU) + s * (N * TAU)
                            + g * GI * TAU,
                            [[LFREE, 48], [TAU, GI], [1, TAU]],
                        )
                        csrc = _ap(pt[:], 0, [[GI * TAU, 48], [TAU, GI], [1, TAU]])
                        if g % 2 == 0:
                            nc.vector.tensor_copy(dst, csrc)
                        else:
                            nc.scalar.copy(dst, csrc)

        # ---- packed matmuls + int8 quantize + store ---------------------
        groups = [(tg * 4, min(4, TAU - tg * 4)) for tg in range((TAU + 3) // 4)]
        for b in range(BL):
            for t0, tlen in groups:
                mt = mm_pool.tile([96, 4 * H], F32, tag="mm")
                for k in range(tlen):
                    tau = t0 + k
                    w_ap = _ap(
                        lhsT[:], b * (2 * N * TAU) + tau, [[LFREE, 128], [TAU, 96]]
                    )
                    r_ap = _ap(
                        hid_sb[:], b * (TAU * H) + tau * H, [[HF, 128], [1, H]]
                    )
                    nc.tensor.matmul(
                        mt[:, k * H : (k + 1) * H], w_ap, r_ap,
                        start=True, stop=True,
                    )
                mt_view = _ap(mt[:], 0, [[4 * H, 96], [H, tlen], [1, H]])
                # per-(row, tau) amax over H -> osc (fp16, also the host scale)
                osc_slice = _ap(osc_sb[:], b * TAU + t0, [[BL * TAU, 96], [1, tlen]])
                nc.vector.reduce_max(
                    osc_slice, mt_view,
                    axis=mybir.AxisListType.X, apply_absolute_value=True,
                )
                rec = rc_pool.tile([96, 4], F32, tag="rc")
                rec_ap = _ap(rec[:], 0, [[4, 96], [1, tlen]])
                nc.vector.scalar_tensor_tensor(
                    rec_ap, osc_slice, 1.0 / 127.0,
                    _ap(tiny[:], 0, [[1, 96], [0, tlen]]), ALU.mult, ALU.add
                )
                nc.vector.reciprocal(out=rec_ap, in_=rec_ap)
                q8 = ot_pool.tile([96, 4 * H], I8, tag="ot")
                q8_view = _ap(q8[:], 0, [[4 * H, 96], [H, tlen], [1, H]])
                nc.vector.tensor_tensor(
                    q8_view, mt_view,
                    _ap(rec[:], 0, [[4, 96], [1, tlen], [0, H]]), ALU.mult
                )
                # scatter to native [b, i, t=2*tau+s, h]; partitions p=(s,i)
                for s in range(2):
                    dst = _dap(
                        out_q, b * N * TP * H + s * H + t0 * 2 * H,
                        [[TP * H, N], [2 * H, tlen], [1, H]],
                    )
                    qsrc = _ap(
                        q8[:], (s * N) * (4 * H), [[4 * H, N], [H, tlen], [1, H]]
                    )
                    nc.sync.dma_start(dst, qsrc)
        # pack fp16 amax scales into out rows T (s=0) / T+1 (s=1):
        # out_q[b, i, T+s, 0:2*TAU] <- bytes of osc_sb[s*48+i, b*TAU : (b+1)*TAU]
        out16 = out_q.bitcast(F16)          # [BL, N, TP, H//2] view
        H2 = H // 2
        for s in range(2):
            for b in range(BL):
                src = _ap(
                    osc_sb[:], (s * N) * (BL * TAU) + b * TAU,
                    [[BL * TAU, N], [1, TAU]],
                )
                dst = _dap(
                    out16, b * N * TP * H2 + (T + s) * H2,
                    [[TP * H2, N], [1, TAU]],
                )
                nc.sync.dma_start(dst, src)


# ----------------------------------------------------------------------------
# Host side
# ----------------------------------------------------------------------------

_RUN = None
_POOL = None
_FPOOL = None
LAST_EXEC_NS = None


def _pool():
    global _POOL
    if _POOL is None:
        _POOL = ThreadPoolExecutor(max_workers=8)
    return _POOL


def _fetch_pool():
    global _FPOOL
    if _FPOOL is None:
        _FPOOL = ThreadPoolExecutor(max_workers=NCHUNK)
    return _FPOOL


_SPOOL = None


def _shard_pool():
    global _SPOOL
    if _SPOOL is None:
        _SPOOL = ThreadPoolExecutor(max_workers=2 * NCORES)
    return _SPOOL


def _build_run():
    global _RUN
    if _RUN is not None:
        return _RUN
    import jax
    from jax.sharding import Mesh, PartitionSpec
    from jax.experimental.shard_map import shard_map
    from concourse.bass2jax import (
        _bass_exec_p,
        fast_dispatch_compile,
        install_neuronx_cc_hook,
        partition_id_tensor,
    )

    nc = build_nc()
    install_neuronx_cc_hook()

    partition_name = (
        nc.partition_id_tensor.name if nc.partition_id_tensor is not None else None
    )
    in_names, out_names, out_avals = [], [], []
    for alloc in nc.m.functions[0].allocations:
        if not isinstance(alloc, mybir.MemoryLocationSet):
            continue
        name = alloc.memorylocations[0].name
        if alloc.kind == "ExternalInput":
            if name != partition_name:
                in_names.append(name)
        elif alloc.kind == "ExternalOutput":
            out_names.append(name)
            out_avals.append(
                jax.core.ShapedArray(
                    tuple(alloc.tensor_shape), mybir.dt.np(alloc.dtype)
                )
            )
    bind_names = list(in_names) + ([partition_name] if partition_name else [])

    def _body(*args):
        operands = list(args)
        if partition_name is not None:
            operands.append(partition_id_tensor())
        outs = _bass_exec_p.bind(
            *operands,
            out_avals=tuple(out_avals),
            in_names=tuple(bind_names),
            out_names=tuple(out_names),
            lowering_input_output_aliases=(),
            sim_require_finite=True,
            sim_require_nnan=True,
            nc=nc,
        )
        return tuple(outs)

    devices = jax.devices()[:NCORES]
    mesh = Mesh(np.asarray(devices), ("core",))

    shapes = {}
    for alloc in nc.m.functions[0].allocations:
        if not isinstance(alloc, mybir.MemoryLocationSet):
            continue
        name = alloc.memorylocations[0].name
        if name in in_names:
            shp = list(alloc.tensor_shape)
            shp[0] *= NCORES
            shapes[name] = jax.ShapeDtypeStruct(
                tuple(shp), mybir.dt.np(alloc.dtype)
            )

    def _compile():
        jitted = jax.jit(
            shard_map(
                _body,
                mesh=mesh,
                in_specs=(PartitionSpec("core"),) * len(in_names),
                out_specs=(PartitionSpec("core"),) * len(out_names),
                check_rep=False,
            )
        )
        return jitted.lower(*[shapes[n] for n in in_names]).compile()

    sharded = fast_dispatch_compile(_compile)
    _RUN = (sharded, in_names, out_names)
    return _RUN


def _quant_chunk(locs_c, hidden_c):
    """locs_c [CB,N,T,2] f32, hidden_c [CB,N,T,H] f32 -> device input map."""
    # int8-quantize hidden with per-(b,n,t) scale amax/127 (threaded over batch)
    amax = np.empty((CB, N, T), np.float32)
    hidq = np.empty((CB, N, T, H), np.int8)

    def _qb(b):
        a = np.abs(hidden_c[b]).max(-1)
        amax[b] = a
        tmp = hidden_c[b] * (127.0 / (a + 1e-30))[..., None]
        np.rint(tmp, out=tmp)
        hidq[b] = tmp.astype(np.int8)

    list(_pool().map(_qb, range(CB)))

    # coords: per-core [2(coord), 128(s*64+tau), BL*N]
    l6 = locs_c.reshape(NCORES, BL, N, TAU, 2, 2)       # c b n tau s coord
    xtz = np.zeros((NCORES, 2, 2, 64, BL, N), np.float32)
    xtz[:, :, :, :TAU] = l6.transpose(0, 5, 4, 3, 1, 2)
    # filler rows: spread points (x=n, y=0) so junk weights stay finite
    xtz[:, 0, :, TAU:] = np.arange(N, dtype=np.float32)
    xt = xtz.reshape(CB * 2 // BL, 128, BL * N)

    # hidden scales for the device: rows (s,tau), cols (b,j), value amax/127
    s6 = (amax * (1.0 / 127.0)).astype(np.float16)
    s6 = s6.reshape(NCORES, BL, N, TAU, 2)              # c b j tau s
    hscz = np.zeros((NCORES, 2, 64, BL, N), np.float16)
    hscz[:, :, :TAU] = s6.transpose(0, 4, 3, 1, 2)
    hsc = hscz.reshape(NCORES * 128, BL * N)
    return {"xt": xt, "hidq": hidq, "hsc": hsc}


def _dequant_chunk(buf, out_c):
    """buf [CB,N,TP,H] int8 (incl. packed scales) -> out_c [CB,N,T,H] f32."""
    scb = np.ascontiguousarray(buf[:, :, T : T + 2, : TAU * 2])
    sc = scb.view(np.float16).astype(np.float32)        # [CB, N, 2(s), TAU]
    sc = sc.transpose(0, 1, 3, 2).reshape(CB, N, T) * (1.0 / 127.0)

    def _db(b):
        out_c[b] = buf[b, :, :T, :]
        out_c[b] *= sc[b][..., None]

    list(_pool().map(_db, range(CB)))


def kernel(locs, hidden, rel_rec=None, rel_send=None):
    locs = np.asarray(locs, dtype=np.float32)
    hidden = np.asarray(hidden, dtype=np.float32)
    sharded, in_names, out_names = _build_run()

    outs = []
    for k in range(NCHUNK):
        ins = _quant_chunk(
            locs[k * CB : (k + 1) * CB], hidden[k * CB : (k + 1) * CB]
        )
        outs.append(sharded(*[ins[n] for n in in_names]))

    out = np.empty((B, N, T, H), np.float32)

    def _fetch_deq(k):
        # fetch the chunk's shards concurrently; dequant each batch as it lands
        futs = [
            (s.index[0].start, _shard_pool().submit(np.asarray, s.data))
            for s in outs[k][0].addressable_shards
        ]
        for row, fut in futs:
            buf = fut.result().reshape(BL * N, TP, H)   # one batch (BL=1)
            scb = np.ascontiguousarray(buf[:, T : T + 2, : TAU * 2])
            sc = scb.view(np.float16).astype(np.float32)    # [N, 2(s), TAU]
            sc = sc.transpose(0, 2, 1).reshape(N, T) * (1.0 / 127.0)
            np.multiply(buf[:, :T, :], sc[..., None], out=out[k * CB + row])

    list(_fetch_pool().map(_fetch_deq, range(NCHUNK)))
    return out


if __name__ == "__main__":
    # smoke test: two different random datasets against a local numpy reference
    rng = np.random.default_rng(0)
    for trial in range(2):
        locs = rng.standard_normal((B, N, T, 2), dtype=np.float32)
        hidden = rng.standard_normal((B, N, T, H), dtype=np.float32)
        got = kernel(locs, hidden)
        x = locs[..., 0]
        y = locs[..., 1]
        d = np.sqrt(
            (x[:, :, None] - x[:, None]) ** 2 + (y[:, :, None] - y[:, None]) ** 2
        )
        w = 1.0 / (d + EPS) * (1.0 - np.eye(N)[None, :, :, None])
        want = np.einsum("bijt,bjth->bith", w.astype(np.float32), hidden)
        err = np.linalg.norm(got - want) / np.linalg.norm(want)
        print(f"trial {trial} rel err vs numpy: {err}")
